# revision 1
# baseline (speedup 1.0000x reference)
"""Bass/Trainium2 fused single-launch kernel for nn_BayesianGNN.

Computation (reference):
    agg1 = spmm(x, ew, src, dst)                       # [N, IN]
    for t in range(T):
        h_t   = relu(agg1 @ (W1 * mask1[t]))           # [N, HID]
        agg2  = spmm(h_t, ew, src, dst)                # [N, HID]
        out_t = agg2 @ (W2 * mask2[t])                 # [N, OUT]

Restructure: spmm is linear, so out_t = spmm(relu(agg1 @ W1m_t) @ W2m_t);
all T samples concatenate into P [N, T*OUT=512] so the second spmm runs
once over 512-wide rows.

Single NEFF per core (SPMD over 8 cores, node-sharded):
  phase A: SpMM1 for the core's 49 dst blocks (dma_gather of x rows +
    precomputed one-hot S-matrix matmul segment-sum) + dense per-sample
    MLP -> P shard [6272, 512] -> internal DRAM.
  AllGather collective: P shards -> full P [50176, 512] in shared DRAM.
  phase B: SpMM2: dma_gather of P rows by src + same S matmul -> out
    shard [6272, 512] f32.

The S segment-sum matrices (S[e, j] = w_e * (dst_local_e == j), one
[128, 128] tile per 128-edge chunk) are precomputed on host and streamed
from HBM, replacing the per-chunk on-device one-hot builds.

dma_gather indices are int16, so gather tables are split in two halves of
25088 rows; per (block, half) the edge list is padded to a fixed number of
128-edge chunks (K_lo/K_hi) so the program is identical across all 8
cores, with padding slots carrying zero rows in S.
"""

import sys

if "/opt/trn_rl_repo" not in sys.path:
    sys.path.insert(0, "/opt/trn_rl_repo")

import math

import numpy as np

import concourse.bass as bass
import concourse.tile as tile
from concourse import bacc, mybir
from concourse.bass import ts

F32 = mybir.dt.float32
BF16 = mybir.dt.bfloat16
I16 = mybir.dt.int16
I32 = mybir.dt.int32
USE_BF16 = True
DT = BF16 if USE_BF16 else F32

N, E = 50000, 800000
IN, HID, OUT, T = 96, 128, 64, 8
P = 128  # partitions
NCORES = 8
NBLK = 392  # node blocks of 128
NP_ = NBLK * P  # padded node count 50176
BPC = NBLK // NCORES  # blocks per core = 49
NPC = BPC * P  # nodes per core = 6272
HALF = NP_ // 2  # gather table half size = 25088
TO = T * OUT  # 512


def _np_dt():
    if USE_BF16:
        import ml_dtypes

        return np.dtype(ml_dtypes.bfloat16)
    return np.dtype(np.float32)


def _pad_table(a, cols):
    """[n, c] -> zero-padded [NP_, cols], split into (lo, hi) halves."""
    out = np.zeros((NP_, cols), _np_dt())
    out[: a.shape[0], : a.shape[1]] = a
    return out[:HALF], out[HALF:]


# --------------------------------------------------------------------------
# host-side graph prep
# --------------------------------------------------------------------------
def prep_graph(src, dst, ew):
    """Partition + pad edges into per-(core, block, half) chunk schedules.

    Returns (K_lo, K_hi, per_core, cmin): per_core[c] holds the int16
    gather index arrays, int32 counts, and the bf16 S-matrix table
    s_tab [P, BPC*(K_lo+K_hi)*P] in processing order (block-major,
    lo chunks then hi chunks).
    """
    src = np.asarray(src).astype(np.int64).ravel()
    dst = np.asarray(dst).astype(np.int64).ravel()
    ew = np.asarray(ew, dtype=np.float32).ravel()

    blk = dst >> 7
    half = (src >= HALF).astype(np.int64)
    order = np.lexsort((src, half, blk))
    sblk = blk[order]
    shalf = half[order]
    ssrc = src[order]
    sew = ew[order]
    sdl = (dst[order] & 127).astype(np.int64)

    cell = sblk * 2 + shalf
    counts = np.bincount(cell, minlength=NBLK * 2)
    K_lo = int(math.ceil(counts[0::2].max() / P))
    K_hi = int(math.ceil(counts[1::2].max() / P))
    KT = K_lo + K_hi

    cell_starts = np.zeros(NBLK * 2 + 1, np.int64)
    np.cumsum(counts, out=cell_starts[1:])
    pos = np.arange(E, dtype=np.int64) - cell_starts[cell]

    b_local = sblk % BPC
    core = sblk // BPC

    per_core = []
    for c in range(NCORES):
        d = {}
        sflat = np.zeros((BPC * KT * P, P), np.float32)
        for s, K, off_h, tag in ((0, K_lo, 0, "lo"), (1, K_hi, K_lo, "hi")):
            m = (core == c) & (shalf == s)
            nslot = BPC * K * P
            idxf = np.full(nslot, -1, np.int16)
            slots = b_local[m] * (K * P) + pos[m]
            idxf[slots] = (ssrc[m] - s * HALF).astype(np.int16)
            cnt = np.bincount(b_local[m], minlength=BPC).astype(np.int32)
            # never let a gather have zero valid indices
            empty = cnt == 0
            if empty.any():
                for b in np.nonzero(empty)[0]:
                    idxf[b * K * P] = 0
                cnt[empty] = 1
            d["idx_" + tag] = np.ascontiguousarray(
                np.tile(idxf.reshape(BPC * K * 8, 16).T, (8, 1))
            )
            d["cnt_" + tag] = np.ascontiguousarray(cnt.reshape(1, BPC))
            # S rows: tile index = b_local*KT + off_h + k, partition = e
            k_idx = pos[m] // P
            e_idx = pos[m] % P
            rows = (b_local[m] * KT + off_h + k_idx) * P + e_idx
            sflat[rows, sdl[m]] = sew[m]
        d["s_tab"] = np.ascontiguousarray(
            sflat.reshape(BPC * KT, P, P)
            .transpose(1, 0, 2)
            .reshape(P, BPC * KT * P)
            .astype(_np_dt())
        )
        per_core.append(d)
    cmin = {}
    for tag in ("lo", "hi"):
        allcnt = np.stack([pc["cnt_" + tag][0] for pc in per_core])  # [NCORES, BPC]
        cmin[tag] = (allcnt.min(axis=0) // P).astype(int).tolist()
    return K_lo, K_hi, per_core, cmin


# --------------------------------------------------------------------------
# fused kernel: SpMM1 + MLP -> AllGather(P) -> SpMM2
# --------------------------------------------------------------------------
def build_kernel_fused(K_lo, K_hi, cmin):
    KT = K_lo + K_hi
    NQ = 4  # SWDGE queues; queue = global gather index % NQ (sem-lane parity)
    nc = bacc.Bacc(
        "TRN2",
        target_bir_lowering=False,
        debug=False,
        num_devices=NCORES,
        num_swdge_queues=NQ,
        dynamic_dma_scratch_size=1 << 16,
    )

    x_lo = nc.dram_tensor("x_lo", [HALF, P], DT, kind="ExternalInput")
    x_hi = nc.dram_tensor("x_hi", [HALF, P], DT, kind="ExternalInput")
    idx_lo = nc.dram_tensor("idx_lo", [P, BPC * K_lo * 8], I16, kind="ExternalInput")
    idx_hi = nc.dram_tensor("idx_hi", [P, BPC * K_hi * 8], I16, kind="ExternalInput")
    cnt_lo = nc.dram_tensor("cnt_lo", [1, BPC], I32, kind="ExternalInput")
    cnt_hi = nc.dram_tensor("cnt_hi", [1, BPC], I32, kind="ExternalInput")
    s_tab = nc.dram_tensor("s_tab", [P, BPC * KT * P], DT, kind="ExternalInput")
    w1 = nc.dram_tensor("w1", [IN, HID], F32, kind="ExternalInput")
    m1 = nc.dram_tensor("m1", [IN, T * HID], F32, kind="ExternalInput")
    w2 = nc.dram_tensor("w2", [HID, OUT], F32, kind="ExternalInput")
    m2 = nc.dram_tensor("m2", [HID, T * OUT], F32, kind="ExternalInput")
    oshard = nc.dram_tensor("oshard", [NPC, TO], F32, kind="ExternalOutput")
    p_int = nc.dram_tensor("p_int", [NPC, TO], DT, kind="Internal")
    p_full = nc.dram_tensor(
        "p_full", [NP_, TO], DT, kind="Internal", addr_space="Shared"
    )

    with tile.TileContext(nc) as tc:
        with tc.tile_pool(name="const", bufs=1) as cpool:
            # ---- load constants
            def load(t_dram, shape, dtype=F32):
                nm = f"c_{t_dram.name}"
                t_sb = cpool.tile([P, shape[1]], dtype, name=nm, tag=nm)
                nc.sync.dma_start(out=t_sb[: shape[0], :], in_=t_dram[:])
                return t_sb

            idx_lo_t = load(idx_lo, [P, BPC * K_lo * 8], I16)
            idx_hi_t = load(idx_hi, [P, BPC * K_hi * 8], I16)
            cnt_lo_t = load(cnt_lo, [1, BPC], I32)
            cnt_hi_t = load(cnt_hi, [1, BPC], I32)
            ztail = max(
                [(K_lo - c) for c in cmin["lo"]] + [(K_hi - c) for c in cmin["hi"]]
            )
            zero_t = cpool.tile([P, ztail * TO], DT, name="zero_t", tag="zero_t")
            nc.vector.memset(zero_t[:], 0.0)
            creg = {
                "lo": nc.gpsimd.alloc_register("cnt_reg_lo"),
                "hi": nc.gpsimd.alloc_register("cnt_reg_hi"),
            }
            cnt_tiles = {"lo": cnt_lo_t, "hi": cnt_hi_t}
            w1_t = load(w1, [IN, HID])
            m1_t = load(m1, [IN, T * HID])
            w2_t = load(w2, [HID, OUT])
            m2_t = load(m2, [HID, T * OUT])

            # masked weights; rows IN..P of w1m stay zero
            w1m = cpool.tile([P, T * HID], DT)
            nc.gpsimd.memset(w1m[:], 0.0)
            for t in range(T):
                nc.vector.tensor_tensor(
                    out=w1m[:IN, ts(t, HID)],
                    in0=w1_t[:IN, :],
                    in1=m1_t[:IN, ts(t, HID)],
                    op=mybir.AluOpType.mult,
                )
            w2m = cpool.tile([P, T * OUT], DT)
            for t in range(T):
                nc.vector.tensor_tensor(
                    out=w2m[:, ts(t, OUT)],
                    in0=w2_t[:, :],
                    in1=m2_t[:, ts(t, OUT)],
                    op=mybir.AluOpType.mult,
                )

            # agg1 transposed [feat, node] for the whole shard, kept in SBUF
            agg1t = cpool.tile([P, NPC], DT)

            # ---------------- phase A: SpMM1 + dense MLP ----------------
            with (
                tc.tile_pool(name="sA", bufs=4) as sApool,
                tc.tile_pool(name="glo", bufs=5) as gpool,
                tc.tile_pool(name="h", bufs=3) as hpool,
                tc.tile_pool(name="po", bufs=2) as ppool,
                tc.tile_pool(name="acc", bufs=2, space="PSUM") as acc_pool,
                tc.tile_pool(name="ph", bufs=2, space="PSUM") as ph_pool,
                tc.tile_pool(name="pp", bufs=1, space="PSUM") as pp_pool,
            ):
                halves_a = (
                    (K_lo, x_lo, idx_lo_t, "lo", 0, 0),
                    (K_hi, x_hi, idx_hi_t, "hi", K_lo, 1),
                )

                def spmm1_block(b):
                    s_blk = sApool.tile([P, KT * P], DT, tag="sA", name="s_blk")
                    nc.sync.dma_start(
                        out=s_blk[:], in_=s_tab[:, b * KT * P : (b + 1) * KT * P]
                    )
                    acc = acc_pool.tile([P, P], F32, space="PSUM", tag="acc", name="acc")
                    i_mm = 0
                    for K, xtab, idx_t, hkey, off_h, hq in halves_a:
                        g = gpool.tile([P, K * P], DT, tag="g" + hkey, name="g" + hkey)
                        cm = cmin[hkey][b]
                        if cm < K:
                            nc.scalar.activation(
                                out=g[:, cm * P : K * P],
                                in_=zero_t[:, : (K - cm) * P],
                                func=mybir.ActivationFunctionType.Relu,
                            )
                        nreg = creg[hkey]
                        nc.gpsimd.reg_load(nreg, cnt_tiles[hkey][0:1, b : b + 1])
                        nc.gpsimd.dma_gather(
                            g[:].rearrange("p (k e) -> p k e", e=P),
                            xtab[:],
                            idx_t[:, b * K * 8 : (b + 1) * K * 8],
                            K * P,
                            nreg,
                            P,
                            single_packet=False,
                            queue_num=(2 * b + hq) % NQ,
                        )
                        for k in range(K):
                            nc.tensor.matmul(
                                out=acc[:],
                                lhsT=g[:, ts(k, P)],
                                rhs=s_blk[:, ts(off_h + k, P)],
                                start=(i_mm == 0),
                                stop=(i_mm == KT - 1),
                            )
                            i_mm += 1
                    nc.vector.tensor_copy(out=agg1t[:, ts(b, P)], in_=acc[:])

                def make_dense_slices(off, w_):
                    """Per-sample slices of one dense 512-node tile, drained a
                    few at a time between spmm blocks so the MLP's LDWEIGHTS-
                    heavy matmul bursts don't monopolize TensorE and starve
                    the gather pipeline of free g buffers."""
                    nj = w_ // P
                    state = {}

                    def make_t(t):
                        def emit_t():
                            if t == 0:
                                state["psum_p"] = [
                                    pp_pool.tile(
                                        [P, TO], F32, space="PSUM",
                                        tag=f"pp{j}", name=f"pp{j}",
                                    )
                                    for j in range(nj)
                                ]
                            psum_h = ph_pool.tile(
                                [P, w_], F32, space="PSUM", tag="ph", name="ph"
                            )
                            nc.tensor.matmul(
                                out=psum_h[:],
                                lhsT=w1m[:, ts(t, HID)],
                                rhs=agg1t[:, off : off + w_],
                                start=True,
                                stop=True,
                            )
                            h_sb = hpool.tile([P, w_], DT, tag="h", name="h_sb")
                            nc.scalar.activation(
                                out=h_sb[:],
                                in_=psum_h[:],
                                func=mybir.ActivationFunctionType.Relu,
                            )
                            for j in range(nj):
                                nc.tensor.matmul(
                                    out=state["psum_p"][j][:, ts(t, OUT)],
                                    lhsT=h_sb[:, ts(j, P)],
                                    rhs=w2m[:, ts(t, OUT)],
                                    start=True,
                                    stop=True,
                                )

                        return emit_t

                    def drain():
                        for j in range(nj):
                            p_sb = ppool.tile([P, TO], DT, tag="po", name="p_sb")
                            nc.scalar.copy(out=p_sb[:], in_=state["psum_p"][j][:])
                            nc.sync.dma_start(
                                out=p_int[off + j * P : off + (j + 1) * P, :],
                                in_=p_sb[:],
                            )

                    return [make_t(t) for t in range(T)] + [drain]

                # interleave: enqueue each dense tile's slices once its agg1t
                # columns are complete; drain up to 3 slices per spmm block
                st_widths = []
                off = 0
                while off < NPC:
                    w_ = min(512, NPC - off)
                    st_widths.append((off, w_))
                    off += w_
                dense_queue = []
                b = 0
                for off, w_ in st_widths:
                    while b * P < off + w_:
                        spmm1_block(b)
                        b += 1
                        for _ in range(3):
                            if dense_queue:
                                dense_queue.pop(0)()
                    dense_queue.extend(make_dense_slices(off, w_))
                while dense_queue:
                    dense_queue.pop(0)()

            # ---------------- AllGather of P shards ----------------
            nc.gpsimd.collective_compute(
                "AllGather",
                mybir.AluOpType.bypass,
                replica_groups=[list(range(NCORES))],
                ins=[p_int[:].opt()],
                outs=[p_full[:].opt()],
            )

            # ---------------- phase B: SpMM2 over P ----------------
            with (
                tc.tile_pool(name="sB", bufs=4) as sBpool,
                tc.tile_pool(name="g2", bufs=4) as g2pool,
                tc.tile_pool(name="o", bufs=3) as opool,
                tc.tile_pool(name="po2", bufs=6, space="PSUM") as po2_pool,
            ):
                halves_b = (
                    (K_lo, 0, idx_lo_t, "lo", 0, 0),
                    (K_hi, HALF, idx_hi_t, "hi", K_lo, 1),
                )
                for b in range(BPC):
                    s_blk = sBpool.tile([P, KT * P], DT, tag="sB", name="s_blk2")
                    nc.sync.dma_start(
                        out=s_blk[:], in_=s_tab[:, b * KT * P : (b + 1) * KT * P]
                    )
                    acc = po2_pool.tile([P, TO], F32, space="PSUM", tag="acc2")
                    i_mm = 0
                    for K, row0, idx_t, hkey, off_h, hq in halves_b:
                        g = g2pool.tile([P, K * TO], DT, tag="g2" + hkey)
                        cm = cmin[hkey][b]
                        if cm < K:
                            nc.scalar.activation(
                                out=g[:, cm * TO : K * TO],
                                in_=zero_t[:, : (K - cm) * TO],
                                func=mybir.ActivationFunctionType.Relu,
                            )
                        nreg = creg[hkey]
                        nc.gpsimd.reg_load(nreg, cnt_tiles[hkey][0:1, b : b + 1])
                        nc.gpsimd.dma_gather(
                            g[:].rearrange("p (k e) -> p k e", e=TO),
                            p_full[row0 : row0 + HALF, :],
                            idx_t[:, b * K * 8 : (b + 1) * K * 8],
                            K * P,
                            nreg,
                            TO,
                            single_packet=False,
                            queue_num=(2 * BPC + 2 * b + hq) % NQ,
                        )
                        for k in range(K):
                            nc.tensor.matmul(
                                out=acc[:],
                                lhsT=s_blk[:, ts(off_h + k, P)],
                                rhs=g[:, ts(k, TO)],
                                start=(i_mm == 0),
                                stop=(i_mm == KT - 1),
                            )
                            i_mm += 1
                    o_sb = opool.tile([P, TO], F32, tag="o")
                    nc.vector.tensor_copy(out=o_sb[:], in_=acc[:])
                    nc.sync.dma_start(out=oshard[ts(b, P), :], in_=o_sb[:])

    nc.compile()
    return nc


# --------------------------------------------------------------------------
# host orchestration
# --------------------------------------------------------------------------
def prep_inputs(x, edge_weight, W1, W2, mask1, mask2, src, dst):
    K_lo, K_hi, per_core, cmin = prep_graph(src, dst, edge_weight)
    x_lo, x_hi = _pad_table(np.asarray(x, np.float32), P)
    w1 = np.ascontiguousarray(np.asarray(W1, np.float32))
    w2 = np.ascontiguousarray(np.asarray(W2, np.float32))
    m1 = np.ascontiguousarray(
        np.asarray(mask1, np.float32).transpose(1, 0, 2).reshape(IN, T * HID)
    )
    m2 = np.ascontiguousarray(
        np.asarray(mask2, np.float32).transpose(1, 0, 2).reshape(HID, T * OUT)
    )

    in_maps = []
    for c in range(NCORES):
        m = dict(per_core[c])
        m.update(x_lo=x_lo, x_hi=x_hi, w1=w1, m1=m1, w2=w2, m2=m2)
        in_maps.append(m)
    return K_lo, K_hi, cmin, per_core, in_maps


def assemble_output(oshards):
    full = np.concatenate(oshards, axis=0)  # [NP_, 512]
    return np.ascontiguousarray(
        full[:N].reshape(N, T, OUT).transpose(1, 0, 2)
    ).astype(np.float32)


def kernel(x, edge_weight, W1, W2, mask1, mask2, src, dst):
    from concourse.bass_utils import run_bass_kernel_spmd

    K_lo, K_hi, cmin, per_core, in_maps = prep_inputs(
        x, edge_weight, W1, W2, mask1, mask2, src, dst
    )
    nc = build_kernel_fused(K_lo, K_hi, cmin)
    res = run_bass_kernel_spmd(nc, in_maps, core_ids=list(range(NCORES)))
    oshards = [res.results[c]["oshard"] for c in range(NCORES)]
    return assemble_output(oshards)



# revision 5
# speedup vs baseline: 55.6919x; 55.6919x over previous
"""Bass/Trainium2 fused single-launch kernel for nn_BayesianGNN.

Computation (reference):
    agg1 = spmm(x, ew, src, dst)                       # [N, IN]
    for t in range(T):
        h_t   = relu(agg1 @ (W1 * mask1[t]))           # [N, HID]
        agg2  = spmm(h_t, ew, src, dst)                # [N, HID]
        out_t = agg2 @ (W2 * mask2[t])                 # [N, OUT]

Restructure: spmm is linear, so out_t = spmm(relu(agg1 @ W1m_t) @ W2m_t);
all T samples concatenate into P [N, T*OUT=512] so the second spmm runs
once over 512-wide rows.

Single NEFF per core (SPMD over 8 cores, dst-node-sharded):
  phase A: SpMM1 over host-pre-gathered x rows (xg, edge-slot order --
    pure streaming, no on-device gather) x on-device-built one-hot
    segment-sum matrices S -> agg1t in SBUF; dense per-sample MLP ->
    P shard rows, drained to 4 row-slice DRAM buffers.
  sliced AllGather: each slice's collective fires as soon as its rows
    are drained, overlapping the halo exchange with phase A compute.
  phase B: SpMM2: dma_gather of P rows by src (SWDGE) + the same S
    matmuls -> out shard [6272, 512] f32.

S matrices (S[e, j] = w_e * (dst_local_e == j)) are built on device per
block with two DVE ops (iota==dst compare, *w) from 2-byte/edge dst and
weight tables, replacing the 256-byte/edge host s_tab of the previous
version.  Gather tails (padding slots) are never zero-filled: pool
buffers are memset once and padding columns of S carry w=0, so stale
(finite) data in the tail contributes exactly zero.

dma_gather indices are int16, so the P table is split in two halves of
25088 rows; per (block, half) the edge list is padded to a fixed number
of 128-edge chunks (K_lo/K_hi) so the program is identical across all 8
cores (counts/indices/gathered-x differ only as input data).
"""

import sys

if "/opt/trn_rl_repo" not in sys.path:
    sys.path.insert(0, "/opt/trn_rl_repo")

import math

import numpy as np

import concourse.bass as bass
import concourse.tile as tile
from concourse import bacc, mybir
from concourse.bass import ts

F32 = mybir.dt.float32
BF16 = mybir.dt.bfloat16
I16 = mybir.dt.int16
I32 = mybir.dt.int32
USE_BF16 = True
DT = BF16 if USE_BF16 else F32

N, E = 50000, 800000
IN, HID, OUT, T = 96, 128, 64, 8
P = 128  # partitions
NCORES = 8
NBLK = 392  # node blocks of 128
NP_ = NBLK * P  # padded node count 50176
BPC = NBLK // NCORES  # blocks per core = 49
NPC = BPC * P  # nodes per core = 6272
HALF = NP_ // 2  # gather table half size = 25088
TO = T * OUT  # 512
NQ = 1  # SWDGE queues (single queue keeps sem-lane/queue binding consistent)

# p_int row slices (per-core local row ranges); collectives fire per slice.
SL_START = [0, 2048, 4096, 6144]
SL_LEN = [2048, 2048, 2048, 128]
NSLICE = len(SL_LEN)
# global p_full base row of each slice block
SL_BASE = [0]
for s in range(1, NSLICE):
    SL_BASE.append(SL_BASE[-1] + NCORES * SL_LEN[s - 1])


def _np_dt():
    if USE_BF16:
        import ml_dtypes

        return np.dtype(ml_dtypes.bfloat16)
    return np.dtype(np.float32)


def _prow_of_node():
    """Map global padded node id -> row in the slice-ordered p_full."""
    n = np.arange(NP_, dtype=np.int64)
    c = n // NPC
    r = n % NPC
    s = np.searchsorted(np.asarray(SL_START + [NPC]), r, side="right") - 1
    base = np.asarray(SL_BASE)[s]
    ln = np.asarray(SL_LEN)[s]
    st = np.asarray(SL_START)[s]
    return base + c * ln + (r - st)


# --------------------------------------------------------------------------
# host-side graph prep
# --------------------------------------------------------------------------
def prep_graph(src, dst, ew, x):
    """Partition + pad edges into per-(core, block, half) chunk schedules.

    Returns (K_lo, K_hi, per_core): per_core[c] holds the int16 gather
    index arrays + int32 counts (phase B), bf16 dst/weight tables (S
    build), and the bf16 pre-gathered x rows xg [P, BPC*KT*96] in edge
    slot order (phase A streaming).
    """
    src = np.asarray(src).astype(np.int64).ravel()
    dst = np.asarray(dst).astype(np.int64).ravel()
    ew = np.asarray(ew, dtype=np.float32).ravel()
    x = np.asarray(x, dtype=np.float32)

    prow_map = _prow_of_node()
    pf = prow_map[src]  # p_full row of each edge's source
    half = (pf >= HALF).astype(np.int64)
    blk = dst >> 7
    order = np.lexsort((pf, half, blk))
    sblk = blk[order]
    shalf = half[order]
    ssrc = src[order]
    spf = pf[order]
    sew = ew[order]
    sdl = (dst[order] & 127).astype(np.int64)

    cell = sblk * 2 + shalf
    counts = np.bincount(cell, minlength=NBLK * 2)
    K_lo = int(math.ceil(counts[0::2].max() / P))
    K_hi = int(math.ceil(counts[1::2].max() / P))
    KT = K_lo + K_hi

    cell_starts = np.zeros(NBLK * 2 + 1, np.int64)
    np.cumsum(counts, out=cell_starts[1:])
    pos = np.arange(E, dtype=np.int64) - cell_starts[cell]

    b_local = sblk % BPC
    core = sblk // BPC

    per_core = []
    for c in range(NCORES):
        d = {}
        # slot id within this core: (b_local, k_global, p)
        nslot_t = BPC * KT * P
        slot_src = np.full(nslot_t, -1, np.int64)
        slot_dl = np.zeros(nslot_t, np.int64)
        slot_w = np.zeros(nslot_t, np.float32)
        for s, K, off_h, tag in ((0, K_lo, 0, "lo"), (1, K_hi, K_lo, "hi")):
            m = (core == c) & (shalf == s)
            nslot = BPC * K * P
            idxf = np.full(nslot, -1, np.int16)
            slots_h = b_local[m] * (K * P) + pos[m]
            idxf[slots_h] = (spf[m] - s * HALF).astype(np.int16)
            cnt = np.bincount(b_local[m], minlength=BPC).astype(np.int32)
            # never let a gather have zero valid indices
            empty = cnt == 0
            if empty.any():
                for b in np.nonzero(empty)[0]:
                    idxf[b * K * P] = 0
                cnt[empty] = 1
            d["idx_" + tag] = np.ascontiguousarray(
                np.tile(idxf.reshape(BPC * K * 8, 16).T, (8, 1))
            )
            d["cnt_" + tag] = np.ascontiguousarray(cnt.reshape(1, BPC))
            # global slot ids (k axis combines lo then hi chunks)
            k_idx = pos[m] // P
            e_idx = pos[m] % P
            gslots = (b_local[m] * KT + off_h + k_idx) * P + e_idx
            slot_src[gslots] = ssrc[m]
            slot_dl[gslots] = sdl[m]
            slot_w[gslots] = sew[m]
        # dst/weight tables [P, BPC*KT] (partition = edge-in-chunk)
        d["dsttab"] = np.ascontiguousarray(
            slot_dl.reshape(BPC * KT, P).T.astype(_np_dt())
        )
        d["wtab"] = np.ascontiguousarray(
            slot_w.reshape(BPC * KT, P).T.astype(_np_dt())
        )
        # pre-gathered x rows [P, BPC*KT*96]; padding slots -> zero rows
        xg = x[np.clip(slot_src, 0, None)]
        xg[slot_src < 0] = 0.0
        d["xg"] = np.ascontiguousarray(
            xg.reshape(BPC * KT, P, IN)
            .transpose(1, 0, 2)
            .reshape(P, BPC * KT * IN)
            .astype(_np_dt())
        )
        per_core.append(d)
    return K_lo, K_hi, per_core


# --------------------------------------------------------------------------
# fused kernel: SpMM1 + MLP -> sliced AllGather(P) -> SpMM2
# --------------------------------------------------------------------------
def build_kernel_fused(K_lo, K_hi):
    KT = K_lo + K_hi
    nc = bacc.Bacc(
        "TRN2",
        target_bir_lowering=False,
        debug=False,
        num_devices=NCORES,
        num_swdge_queues=NQ,
        dynamic_dma_scratch_size=1 << 16,
    )

    xg = nc.dram_tensor("xg", [P, BPC * KT * IN], DT, kind="ExternalInput")
    dsttab = nc.dram_tensor("dsttab", [P, BPC * KT], DT, kind="ExternalInput")
    wtab = nc.dram_tensor("wtab", [P, BPC * KT], DT, kind="ExternalInput")
    iota = nc.dram_tensor("iota", [P, P], DT, kind="ExternalInput")
    idx_lo = nc.dram_tensor("idx_lo", [P, BPC * K_lo * 8], I16, kind="ExternalInput")
    idx_hi = nc.dram_tensor("idx_hi", [P, BPC * K_hi * 8], I16, kind="ExternalInput")
    cnt_lo = nc.dram_tensor("cnt_lo", [1, BPC], I32, kind="ExternalInput")
    cnt_hi = nc.dram_tensor("cnt_hi", [1, BPC], I32, kind="ExternalInput")
    w1 = nc.dram_tensor("w1", [IN, HID], F32, kind="ExternalInput")
    m1 = nc.dram_tensor("m1", [IN, T * HID], F32, kind="ExternalInput")
    w2 = nc.dram_tensor("w2", [HID, OUT], F32, kind="ExternalInput")
    m2 = nc.dram_tensor("m2", [HID, T * OUT], F32, kind="ExternalInput")
    oshard = nc.dram_tensor("oshard", [NPC, TO], F32, kind="ExternalOutput")
    p_int_s = [
        nc.dram_tensor(f"p_int{s}", [SL_LEN[s], TO], DT, kind="Internal")
        for s in range(NSLICE)
    ]
    p_full = nc.dram_tensor(
        "p_full", [NP_, TO], DT, kind="Internal", addr_space="Shared"
    )

    with tile.TileContext(nc) as tc:
        with tc.tile_pool(name="const", bufs=1) as cpool:
            # ---- load constants
            def load(t_dram, shape, dtype=F32):
                nm = f"c_{t_dram.name}"
                t_sb = cpool.tile([P, shape[1]], dtype, name=nm, tag=nm)
                nc.sync.dma_start(out=t_sb[: shape[0], :], in_=t_dram[:])
                return t_sb

            iota_t = load(iota, [P, P], DT)
            dst_t = load(dsttab, [P, BPC * KT], DT)
            w_t = load(wtab, [P, BPC * KT], DT)
            idx_lo_t = load(idx_lo, [P, BPC * K_lo * 8], I16)
            idx_hi_t = load(idx_hi, [P, BPC * K_hi * 8], I16)
            cnt_lo_t = load(cnt_lo, [1, BPC], I32)
            cnt_hi_t = load(cnt_hi, [1, BPC], I32)
            creg = {
                "lo": nc.gpsimd.alloc_register("cnt_reg_lo"),
                "hi": nc.gpsimd.alloc_register("cnt_reg_hi"),
            }
            cnt_tiles = {"lo": cnt_lo_t, "hi": cnt_hi_t}
            w1_t = load(w1, [IN, HID])
            m1_t = load(m1, [IN, T * HID])
            w2_t = load(w2, [HID, OUT])
            m2_t = load(m2, [HID, T * OUT])

            # masked weights (only rows :IN of w1m are ever read)
            w1m = cpool.tile([P, T * HID], DT)
            for t in range(T):
                nc.vector.tensor_tensor(
                    out=w1m[:IN, ts(t, HID)],
                    in0=w1_t[:IN, :],
                    in1=m1_t[:IN, ts(t, HID)],
                    op=mybir.AluOpType.mult,
                )
            w2m = cpool.tile([P, T * OUT], DT)
            for t in range(T):
                nc.vector.tensor_tensor(
                    out=w2m[:, ts(t, OUT)],
                    in0=w2_t[:, :],
                    in1=m2_t[:, ts(t, OUT)],
                    op=mybir.AluOpType.mult,
                )

            # agg1 transposed [feat, node] for the whole shard, kept in SBUF
            agg1t = cpool.tile([P, NPC], DT)

            iota_b = iota_t[:].rearrange("p (k j) -> p k j", k=1).broadcast_to(
                [P, KT, P]
            )

            def build_s(spool, eqpool, b):
                """S_blk [P, KT*P]: S[e, k, j] = w[e,k] * (dst[e,k] == j)."""
                s_blk = spool.tile([P, KT * P], DT, tag="s_blk", name="s_blk")
                eq = eqpool.tile([P, KT * P], DT, tag="eq", name="eq")
                dst_b = (
                    dst_t[:, b * KT : (b + 1) * KT]
                    .rearrange("p (k j) -> p k j", j=1)
                    .broadcast_to([P, KT, P])
                )
                w_b = (
                    w_t[:, b * KT : (b + 1) * KT]
                    .rearrange("p (k j) -> p k j", j=1)
                    .broadcast_to([P, KT, P])
                )
                nc.vector.tensor_tensor(
                    out=eq[:].rearrange("p (k j) -> p k j", j=P),
                    in0=iota_b,
                    in1=dst_b,
                    op=mybir.AluOpType.is_equal,
                )
                nc.vector.tensor_tensor(
                    out=s_blk[:].rearrange("p (k j) -> p k j", j=P),
                    in0=eq[:].rearrange("p (k j) -> p k j", j=P),
                    in1=w_b,
                    op=mybir.AluOpType.mult,
                )
                return s_blk

            # ---------------- phase A: SpMM1 + dense MLP ----------------
            with (
                tc.tile_pool(name="sA", bufs=3) as sApool,
                tc.tile_pool(name="eqA", bufs=2) as eqApool,
                tc.tile_pool(name="gA", bufs=3) as gApool,
                tc.tile_pool(name="h", bufs=3) as hpool,
                tc.tile_pool(name="po", bufs=2) as ppool,
                tc.tile_pool(name="acc", bufs=2, space="PSUM") as acc_pool,
                tc.tile_pool(name="ph", bufs=2, space="PSUM") as ph_pool,
                tc.tile_pool(name="pp", bufs=1, space="PSUM") as pp_pool,
            ):

                def spmm1_block(b):
                    g = gApool.tile([P, KT * IN], DT, tag="gA", name="gA")
                    nc.sync.dma_start(
                        out=g[:], in_=xg[:, b * KT * IN : (b + 1) * KT * IN]
                    )
                    s_blk = build_s(sApool, eqApool, b)
                    acc = acc_pool.tile([P, P], F32, space="PSUM", tag="acc", name="acc")
                    for k in range(KT):
                        nc.tensor.matmul(
                            out=acc[:IN, :],
                            lhsT=g[:, ts(k, IN)],
                            rhs=s_blk[:, ts(k, P)],
                            start=(k == 0),
                            stop=(k == KT - 1),
                        )
                    nc.vector.tensor_copy(out=agg1t[:IN, ts(b, P)], in_=acc[:IN, :])

                def mlp_group(off, w_):
                    nj = w_ // P
                    psum_p = [
                        pp_pool.tile([P, TO], F32, space="PSUM", tag=f"pp{j}", name=f"pp{j}")
                        for j in range(nj)
                    ]
                    for t in range(T):
                        psum_h = ph_pool.tile([P, w_], F32, space="PSUM", tag="ph", name="ph")
                        nc.tensor.matmul(
                            out=psum_h[:],
                            lhsT=w1m[:IN, ts(t, HID)],
                            rhs=agg1t[:IN, off : off + w_],
                            start=True,
                            stop=True,
                        )
                        h_sb = hpool.tile([P, w_], DT, tag="h", name="h_sb")
                        nc.scalar.activation(
                            out=h_sb[:],
                            in_=psum_h[:],
                            func=mybir.ActivationFunctionType.Relu,
                        )
                        for j in range(nj):
                            nc.tensor.matmul(
                                out=psum_p[j][:, ts(t, OUT)],
                                lhsT=h_sb[:, ts(j, P)],
                                rhs=w2m[:, ts(t, OUT)],
                                start=True,
                                stop=True,
                            )
                    # drain to the slice buffer this group belongs to
                    s = off // 2048
                    for j in range(nj):
                        p_sb = ppool.tile([P, TO], DT, tag="po", name="p_sb")
                        nc.scalar.copy(out=p_sb[:], in_=psum_p[j][:])
                        r0 = off + j * P - SL_START[s]
                        nc.sync.dma_start(
                            out=p_int_s[s][r0 : r0 + P, :], in_=p_sb[:]
                        )

                groups = []
                off = 0
                while off < NPC:
                    w_ = min(512, NPC - off)
                    groups.append((off, w_))
                    off += w_
                b = 0
                done_slices = 0
                for off, w_ in groups:
                    while b * P < off + w_:
                        spmm1_block(b)
                        b += 1
                    mlp_group(off, w_)
                    # fire the collective for any slice fully drained
                    while (
                        done_slices < NSLICE
                        and off + w_ >= SL_START[done_slices] + SL_LEN[done_slices]
                    ):
                        s = done_slices
                        nc.gpsimd.collective_compute(
                            "AllGather",
                            mybir.AluOpType.bypass,
                            replica_groups=[list(range(NCORES))],
                            ins=[p_int_s[s][:].opt()],
                            outs=[
                                p_full[
                                    SL_BASE[s] : SL_BASE[s] + NCORES * SL_LEN[s]
                                ].opt()
                            ],
                        )
                        done_slices += 1
                assert done_slices == NSLICE

            # ---------------- phase B: SpMM2 over P ----------------
            with (
                tc.tile_pool(name="sB", bufs=3) as sBpool,
                tc.tile_pool(name="eqB", bufs=2) as eqBpool,
                tc.tile_pool(name="o", bufs=3) as opool,
                tc.tile_pool(name="po2", bufs=4, space="PSUM") as po2_pool,
            ):
                halves_b = (
                    (K_lo, 0, idx_lo_t, "lo", 0, 0),
                    (K_hi, HALF, idx_hi_t, "hi", K_lo, 1),
                )
                # persistent gather buffers, memset once: gather tails
                # (slots beyond cnt) keep the zeros / stale finite rows,
                # which S's w=0 padding columns nullify in the matmul.
                NG2 = 4  # buffer cycle must align with the queue cycle (sem-lane binding)
                g2bufs = {}
                for K, row0, idx_t, hkey, off_h, hq in halves_b:
                    g2bufs[hkey] = [
                        cpool.tile([P, K * TO], DT, name=f"g2{hkey}{i}", tag=f"g2{hkey}{i}")
                        for i in range(NG2)
                    ]
                    for g in g2bufs[hkey]:
                        nc.vector.memset(g[:], 0.0)
                for b in range(BPC):
                    s_blk = build_s(sBpool, eqBpool, b)
                    acc = po2_pool.tile([P, TO], F32, space="PSUM", tag="acc2")
                    i_mm = 0
                    for K, row0, idx_t, hkey, off_h, hq in halves_b:
                        g = g2bufs[hkey][b % NG2]
                        nreg = creg[hkey]
                        nc.gpsimd.reg_load(nreg, cnt_tiles[hkey][0:1, b : b + 1])
                        nc.gpsimd.dma_gather(
                            g[:].rearrange("p (k e) -> p k e", e=TO),
                            p_full[row0 : row0 + HALF, :],
                            idx_t[:, b * K * 8 : (b + 1) * K * 8],
                            K * P,
                            nreg,
                            TO,
                            single_packet=False,
                            queue_num=0,
                        )
                        for k in range(K):
                            nc.tensor.matmul(
                                out=acc[:],
                                lhsT=s_blk[:, ts(off_h + k, P)],
                                rhs=g[:, ts(k, TO)],
                                start=(i_mm == 0),
                                stop=(i_mm == KT - 1),
                            )
                            i_mm += 1
                    o_sb = opool.tile([P, TO], F32, tag="o")
                    nc.vector.tensor_copy(out=o_sb[:], in_=acc[:])
                    nc.sync.dma_start(out=oshard[ts(b, P), :], in_=o_sb[:])

    nc.compile()
    return nc


# --------------------------------------------------------------------------
# host orchestration
# --------------------------------------------------------------------------
def prep_inputs(x, edge_weight, W1, W2, mask1, mask2, src, dst):
    K_lo, K_hi, per_core = prep_graph(src, dst, edge_weight, x)
    w1 = np.ascontiguousarray(np.asarray(W1, np.float32))
    w2 = np.ascontiguousarray(np.asarray(W2, np.float32))
    m1 = np.ascontiguousarray(
        np.asarray(mask1, np.float32).transpose(1, 0, 2).reshape(IN, T * HID)
    )
    m2 = np.ascontiguousarray(
        np.asarray(mask2, np.float32).transpose(1, 0, 2).reshape(HID, T * OUT)
    )
    iota_h = np.ascontiguousarray(
        np.broadcast_to(np.arange(P, dtype=np.float32), (P, P)).astype(_np_dt())
    )

    in_maps = []
    for c in range(NCORES):
        m = dict(per_core[c])
        m.update(w1=w1, m1=m1, w2=w2, m2=m2, iota=iota_h)
        in_maps.append(m)
    return K_lo, K_hi, per_core, in_maps


def assemble_output(oshards):
    full = np.concatenate(oshards, axis=0)  # [NP_, 512]
    return np.ascontiguousarray(
        full[:N].reshape(N, T, OUT).transpose(1, 0, 2)
    ).astype(np.float32)


def kernel(x, edge_weight, W1, W2, mask1, mask2, src, dst):
    from concourse.bass_utils import run_bass_kernel_spmd

    K_lo, K_hi, per_core, in_maps = prep_inputs(
        x, edge_weight, W1, W2, mask1, mask2, src, dst
    )
    nc = build_kernel_fused(K_lo, K_hi)
    res = run_bass_kernel_spmd(nc, in_maps, core_ids=list(range(NCORES)))
    oshards = [res.results[c]["oshard"] for c in range(NCORES)]
    return assemble_output(oshards)


# revision 7
# speedup vs baseline: 100.8855x; 1.8115x over previous
"""Bass/Trainium2 fused single-launch kernel for nn_BayesianGNN.

Computation (reference):
    agg1 = spmm(x, ew, src, dst)                       # [N, IN]
    for t in range(T):
        h_t   = relu(agg1 @ (W1 * mask1[t]))           # [N, HID]
        agg2  = spmm(h_t, ew, src, dst)                # [N, HID]
        out_t = agg2 @ (W2 * mask2[t])                 # [N, OUT]

Restructure: spmm is linear, so out_t = spmm(relu(agg1 @ W1m_t) @ W2m_t);
all T samples concatenate into P [N, T*OUT=512] so the second spmm runs
once over 512-wide rows.

Single NEFF per core (SPMD over 8 cores, dst-node-sharded):
  phase A: SpMM1 over host-pre-gathered x rows (xg, edge-slot order --
    pure streaming, no on-device gather) x on-device-built one-hot
    segment-sum matrices S -> agg1t in SBUF; dense per-sample MLP ->
    P shard rows, drained to 4 row-slice DRAM buffers.
  sliced AllGather: each slice's collective fires as soon as its rows
    are drained, overlapping the halo exchange with phase A compute.
  phase B: SpMM2: dma_gather of P rows by src (SWDGE) + the same S
    matmuls -> out shard [6272, 512] f32.

S matrices (S[e, j] = w_e * (dst_local_e == j)) are built on device per
block with two DVE ops (iota==dst compare, *w) from 2-byte/edge dst and
weight tables, replacing the 256-byte/edge host s_tab of the previous
version.  Gather tails (padding slots) are never zero-filled: pool
buffers are memset once and padding columns of S carry w=0, so stale
(finite) data in the tail contributes exactly zero.

dma_gather indices are int16, so the P table is split in two halves of
25088 rows; per (block, half) the edge list is padded to a fixed number
of 128-edge chunks (K_lo/K_hi) so the program is identical across all 8
cores (counts/indices/gathered-x differ only as input data).
"""

import sys

if "/opt/trn_rl_repo" not in sys.path:
    sys.path.insert(0, "/opt/trn_rl_repo")

import math

import numpy as np

import concourse.bass as bass
import concourse.tile as tile
from concourse import bacc, mybir
from concourse.bass import ts

F32 = mybir.dt.float32
BF16 = mybir.dt.bfloat16
I16 = mybir.dt.int16
I32 = mybir.dt.int32
USE_BF16 = True
DT = BF16 if USE_BF16 else F32

N, E = 50000, 800000
IN, HID, OUT, T = 96, 128, 64, 8
P = 128  # partitions
NCORES = 8
NBLK = 392  # node blocks of 128
NP_ = NBLK * P  # padded node count 50176
BPC = NBLK // NCORES  # blocks per core = 49
NPC = BPC * P  # nodes per core = 6272
HALF = NP_ // 2  # gather table half size = 25088
TO = T * OUT  # 512
NQ = 4  # SWDGE queues; gather emission order is pinned (nosync chain) so the
# scheduler's DMASW lane round-robin (8 lanes) stays aligned with queue_num%4

# p_int row slices (per-core local row ranges); collectives fire per slice.
SL_START = [0, 2048, 4096, 6144]
SL_LEN = [2048, 2048, 2048, 128]
NSLICE = len(SL_LEN)
# global p_full base row of each slice block
SL_BASE = [0]
for s in range(1, NSLICE):
    SL_BASE.append(SL_BASE[-1] + NCORES * SL_LEN[s - 1])


def _np_dt():
    if USE_BF16:
        import ml_dtypes

        return np.dtype(ml_dtypes.bfloat16)
    return np.dtype(np.float32)


def _prow_of_node():
    """Map global padded node id -> row in the slice-ordered p_full."""
    n = np.arange(NP_, dtype=np.int64)
    c = n // NPC
    r = n % NPC
    s = np.searchsorted(np.asarray(SL_START + [NPC]), r, side="right") - 1
    base = np.asarray(SL_BASE)[s]
    ln = np.asarray(SL_LEN)[s]
    st = np.asarray(SL_START)[s]
    return base + c * ln + (r - st)


# --------------------------------------------------------------------------
# host-side graph prep
# --------------------------------------------------------------------------
def prep_graph(src, dst, ew, x):
    """Partition + pad edges into per-(core, block, half) chunk schedules.

    Returns (K_lo, K_hi, per_core): per_core[c] holds the int16 gather
    index arrays + int32 counts (phase B), bf16 dst/weight tables (S
    build), and the bf16 pre-gathered x rows xg [P, BPC*KT*96] in edge
    slot order (phase A streaming).
    """
    src = np.asarray(src).astype(np.int64).ravel()
    dst = np.asarray(dst).astype(np.int64).ravel()
    ew = np.asarray(ew, dtype=np.float32).ravel()
    x = np.asarray(x, dtype=np.float32)

    prow_map = _prow_of_node()
    pf = prow_map[src]  # p_full row of each edge's source
    half = (pf >= HALF).astype(np.int64)
    blk = dst >> 7
    order = np.lexsort((pf, half, blk))
    sblk = blk[order]
    shalf = half[order]
    ssrc = src[order]
    spf = pf[order]
    sew = ew[order]
    sdl = (dst[order] & 127).astype(np.int64)

    cell = sblk * 2 + shalf
    counts = np.bincount(cell, minlength=NBLK * 2)
    K_lo = int(math.ceil(counts[0::2].max() / P))
    K_hi = int(math.ceil(counts[1::2].max() / P))
    KT = K_lo + K_hi

    cell_starts = np.zeros(NBLK * 2 + 1, np.int64)
    np.cumsum(counts, out=cell_starts[1:])
    pos = np.arange(E, dtype=np.int64) - cell_starts[cell]

    b_local = sblk % BPC
    core = sblk // BPC

    per_core = []
    for c in range(NCORES):
        d = {}
        # slot id within this core: (b_local, k_global, p)
        nslot_t = BPC * KT * P
        slot_src = np.full(nslot_t, -1, np.int64)
        slot_dl = np.zeros(nslot_t, np.int64)
        slot_w = np.zeros(nslot_t, np.float32)
        for s, K, off_h, tag in ((0, K_lo, 0, "lo"), (1, K_hi, K_lo, "hi")):
            m = (core == c) & (shalf == s)
            nslot = BPC * K * P
            idxf = np.full(nslot, -1, np.int16)
            slots_h = b_local[m] * (K * P) + pos[m]
            idxf[slots_h] = (spf[m] - s * HALF).astype(np.int16)
            cnt = np.bincount(b_local[m], minlength=BPC).astype(np.int32)
            # never let a gather have zero valid indices
            empty = cnt == 0
            if empty.any():
                for b in np.nonzero(empty)[0]:
                    idxf[b * K * P] = 0
                cnt[empty] = 1
            d["idx_" + tag] = np.ascontiguousarray(
                np.tile(idxf.reshape(BPC * K * 8, 16).T, (8, 1))
            )
            d["cnt_" + tag] = np.ascontiguousarray(cnt.reshape(1, BPC))
            # global slot ids (k axis combines lo then hi chunks)
            k_idx = pos[m] // P
            e_idx = pos[m] % P
            gslots = (b_local[m] * KT + off_h + k_idx) * P + e_idx
            slot_src[gslots] = ssrc[m]
            slot_dl[gslots] = sdl[m]
            slot_w[gslots] = sew[m]
        # dst/weight tables [P, BPC*KT] (partition = edge-in-chunk)
        d["dsttab"] = np.ascontiguousarray(
            slot_dl.reshape(BPC * KT, P).T.astype(_np_dt())
        )
        d["wtab"] = np.ascontiguousarray(
            slot_w.reshape(BPC * KT, P).T.astype(_np_dt())
        )
        # pre-gathered x rows [P, BPC*KT*96]; padding slots -> zero rows
        xg = x[np.clip(slot_src, 0, None)]
        xg[slot_src < 0] = 0.0
        d["xg"] = np.ascontiguousarray(
            xg.reshape(BPC * KT, P, IN)
            .transpose(1, 0, 2)
            .reshape(P, BPC * KT * IN)
            .astype(_np_dt())
        )
        per_core.append(d)
    return K_lo, K_hi, per_core


# --------------------------------------------------------------------------
# fused kernel: SpMM1 + MLP -> sliced AllGather(P) -> SpMM2
# --------------------------------------------------------------------------
def build_kernel_fused(K_lo, K_hi):
    KT = K_lo + K_hi
    nc = bacc.Bacc(
        "TRN2",
        target_bir_lowering=False,
        debug=False,
        num_devices=NCORES,
        num_swdge_queues=NQ,
        dynamic_dma_scratch_size=1 << 16,
    )

    xg = nc.dram_tensor("xg", [P, BPC * KT * IN], DT, kind="ExternalInput")
    dsttab = nc.dram_tensor("dsttab", [P, BPC * KT], DT, kind="ExternalInput")
    wtab = nc.dram_tensor("wtab", [P, BPC * KT], DT, kind="ExternalInput")
    iota = nc.dram_tensor("iota", [P, P], DT, kind="ExternalInput")
    idx_lo = nc.dram_tensor("idx_lo", [P, BPC * K_lo * 8], I16, kind="ExternalInput")
    idx_hi = nc.dram_tensor("idx_hi", [P, BPC * K_hi * 8], I16, kind="ExternalInput")
    cnt_lo = nc.dram_tensor("cnt_lo", [1, BPC], I32, kind="ExternalInput")
    cnt_hi = nc.dram_tensor("cnt_hi", [1, BPC], I32, kind="ExternalInput")
    w1 = nc.dram_tensor("w1", [IN, HID], F32, kind="ExternalInput")
    m1 = nc.dram_tensor("m1", [IN, T * HID], F32, kind="ExternalInput")
    w2 = nc.dram_tensor("w2", [HID, OUT], F32, kind="ExternalInput")
    m2 = nc.dram_tensor("m2", [HID, T * OUT], F32, kind="ExternalInput")
    oshard = nc.dram_tensor("oshard", [NPC, TO], F32, kind="ExternalOutput")
    p_int_s = [
        nc.dram_tensor(f"p_int{s}", [SL_LEN[s], TO], DT, kind="Internal")
        for s in range(NSLICE)
    ]
    p_full = nc.dram_tensor(
        "p_full", [NP_, TO], DT, kind="Internal", addr_space="Shared"
    )

    with tile.TileContext(nc) as tc:
        with tc.tile_pool(name="const", bufs=1) as cpool:
            # ---- load constants
            def load(t_dram, shape, dtype=F32):
                nm = f"c_{t_dram.name}"
                t_sb = cpool.tile([P, shape[1]], dtype, name=nm, tag=nm)
                nc.sync.dma_start(out=t_sb[: shape[0], :], in_=t_dram[:])
                return t_sb

            iota_t = load(iota, [P, P], DT)
            dst_t = load(dsttab, [P, BPC * KT], DT)
            w_t = load(wtab, [P, BPC * KT], DT)
            idx_lo_t = load(idx_lo, [P, BPC * K_lo * 8], I16)
            idx_hi_t = load(idx_hi, [P, BPC * K_hi * 8], I16)
            cnt_lo_t = load(cnt_lo, [1, BPC], I32)
            cnt_hi_t = load(cnt_hi, [1, BPC], I32)
            creg = {
                "lo": nc.gpsimd.alloc_register("cnt_reg_lo"),
                "hi": nc.gpsimd.alloc_register("cnt_reg_hi"),
            }
            cnt_tiles = {"lo": cnt_lo_t, "hi": cnt_hi_t}
            w1_t = load(w1, [IN, HID])
            m1_t = load(m1, [IN, T * HID])
            w2_t = load(w2, [HID, OUT])
            m2_t = load(m2, [HID, T * OUT])

            # masked weights (only rows :IN of w1m are ever read)
            w1m = cpool.tile([P, T * HID], DT)
            for t in range(T):
                nc.vector.tensor_tensor(
                    out=w1m[:IN, ts(t, HID)],
                    in0=w1_t[:IN, :],
                    in1=m1_t[:IN, ts(t, HID)],
                    op=mybir.AluOpType.mult,
                )
            w2m = cpool.tile([P, T * OUT], DT)
            for t in range(T):
                nc.vector.tensor_tensor(
                    out=w2m[:, ts(t, OUT)],
                    in0=w2_t[:, :],
                    in1=m2_t[:, ts(t, OUT)],
                    op=mybir.AluOpType.mult,
                )

            # agg1 transposed [feat, node] for the whole shard, kept in SBUF
            agg1t = cpool.tile([P, NPC], DT)

            iota_b = iota_t[:].rearrange("p (k j) -> p k j", k=1).broadcast_to(
                [P, KT, P]
            )

            def build_s(spool, eqpool, b):
                """S_blk [P, KT*P]: S[e, k, j] = w[e,k] * (dst[e,k] == j)."""
                s_blk = spool.tile([P, KT * P], DT, tag="s_blk", name="s_blk")
                eq = eqpool.tile([P, KT * P], DT, tag="eq", name="eq")
                dst_b = (
                    dst_t[:, b * KT : (b + 1) * KT]
                    .rearrange("p (k j) -> p k j", j=1)
                    .broadcast_to([P, KT, P])
                )
                w_b = (
                    w_t[:, b * KT : (b + 1) * KT]
                    .rearrange("p (k j) -> p k j", j=1)
                    .broadcast_to([P, KT, P])
                )
                nc.vector.tensor_tensor(
                    out=eq[:].rearrange("p (k j) -> p k j", j=P),
                    in0=iota_b,
                    in1=dst_b,
                    op=mybir.AluOpType.is_equal,
                )
                nc.vector.tensor_tensor(
                    out=s_blk[:].rearrange("p (k j) -> p k j", j=P),
                    in0=eq[:].rearrange("p (k j) -> p k j", j=P),
                    in1=w_b,
                    op=mybir.AluOpType.mult,
                )
                return s_blk

            # ---------------- phase A: SpMM1 + dense MLP ----------------
            with (
                tc.tile_pool(name="sA", bufs=3) as sApool,
                tc.tile_pool(name="eqA", bufs=2) as eqApool,
                tc.tile_pool(name="gA", bufs=3) as gApool,
                tc.tile_pool(name="h", bufs=3) as hpool,
                tc.tile_pool(name="po", bufs=2) as ppool,
                tc.tile_pool(name="acc", bufs=2, space="PSUM") as acc_pool,
                tc.tile_pool(name="ph", bufs=2, space="PSUM") as ph_pool,
                tc.tile_pool(name="pp", bufs=1, space="PSUM") as pp_pool,
            ):

                def spmm1_block(b):
                    g = gApool.tile([P, KT * IN], DT, tag="gA", name="gA")
                    nc.sync.dma_start(
                        out=g[:], in_=xg[:, b * KT * IN : (b + 1) * KT * IN]
                    )
                    s_blk = build_s(sApool, eqApool, b)
                    acc = acc_pool.tile([P, P], F32, space="PSUM", tag="acc", name="acc")
                    for k in range(KT):
                        nc.tensor.matmul(
                            out=acc[:IN, :],
                            lhsT=g[:, ts(k, IN)],
                            rhs=s_blk[:, ts(k, P)],
                            start=(k == 0),
                            stop=(k == KT - 1),
                        )
                    nc.vector.tensor_copy(out=agg1t[:IN, ts(b, P)], in_=acc[:IN, :])

                def mlp_group(off, w_):
                    nj = w_ // P
                    psum_p = [
                        pp_pool.tile([P, TO], F32, space="PSUM", tag=f"pp{j}", name=f"pp{j}")
                        for j in range(nj)
                    ]
                    for t in range(T):
                        psum_h = ph_pool.tile([P, w_], F32, space="PSUM", tag="ph", name="ph")
                        nc.tensor.matmul(
                            out=psum_h[:],
                            lhsT=w1m[:IN, ts(t, HID)],
                            rhs=agg1t[:IN, off : off + w_],
                            start=True,
                            stop=True,
                        )
                        h_sb = hpool.tile([P, w_], DT, tag="h", name="h_sb")
                        nc.scalar.activation(
                            out=h_sb[:],
                            in_=psum_h[:],
                            func=mybir.ActivationFunctionType.Relu,
                        )
                        for j in range(nj):
                            nc.tensor.matmul(
                                out=psum_p[j][:, ts(t, OUT)],
                                lhsT=h_sb[:, ts(j, P)],
                                rhs=w2m[:, ts(t, OUT)],
                                start=True,
                                stop=True,
                            )
                    # drain to the slice buffer this group belongs to
                    s = off // 2048
                    for j in range(nj):
                        p_sb = ppool.tile([P, TO], DT, tag="po", name="p_sb")
                        nc.scalar.copy(out=p_sb[:], in_=psum_p[j][:])
                        r0 = off + j * P - SL_START[s]
                        nc.sync.dma_start(
                            out=p_int_s[s][r0 : r0 + P, :], in_=p_sb[:]
                        )

                groups = []
                off = 0
                while off < NPC:
                    w_ = min(512, NPC - off)
                    groups.append((off, w_))
                    off += w_
                b = 0
                done_slices = 0
                for off, w_ in groups:
                    while b * P < off + w_:
                        spmm1_block(b)
                        b += 1
                    mlp_group(off, w_)
                    # fire the collective for any slice fully drained
                    while (
                        done_slices < NSLICE
                        and off + w_ >= SL_START[done_slices] + SL_LEN[done_slices]
                    ):
                        s = done_slices
                        nc.gpsimd.collective_compute(
                            "AllGather",
                            mybir.AluOpType.bypass,
                            replica_groups=[list(range(NCORES))],
                            ins=[p_int_s[s][:].opt()],
                            outs=[
                                p_full[
                                    SL_BASE[s] : SL_BASE[s] + NCORES * SL_LEN[s]
                                ].opt()
                            ],
                        )
                        done_slices += 1
                assert done_slices == NSLICE

            # ---------------- phase B: SpMM2 over P ----------------
            with (
                tc.tile_pool(name="sB", bufs=3) as sBpool,
                tc.tile_pool(name="eqB", bufs=2) as eqBpool,
                tc.tile_pool(name="o", bufs=3) as opool,
                tc.tile_pool(name="po2", bufs=4, space="PSUM") as po2_pool,
            ):
                halves_b = (
                    (K_lo, 0, idx_lo_t, "lo", 0, 0),
                    (K_hi, HALF, idx_hi_t, "hi", K_lo, 1),
                )
                # persistent gather buffers, memset once: gather tails
                # (slots beyond cnt) keep the zeros / stale finite rows,
                # which S's w=0 padding columns nullify in the matmul.
                NG2 = 4  # buffer cycle must align with the queue cycle (sem-lane binding)
                g2bufs = {}
                for K, row0, idx_t, hkey, off_h, hq in halves_b:
                    g2bufs[hkey] = [
                        cpool.tile([P, K * TO], DT, name=f"g2{hkey}{i}", tag=f"g2{hkey}{i}")
                        for i in range(NG2)
                    ]
                    for g in g2bufs[hkey]:
                        nc.vector.memset(g[:], 0.0)
                from concourse.tile_rust import add_dep_helper

                prev_gather = None
                for b in range(BPC):
                    s_blk = build_s(sBpool, eqBpool, b)
                    acc = po2_pool.tile([P, TO], F32, space="PSUM", tag="acc2")
                    i_mm = 0
                    for K, row0, idx_t, hkey, off_h, hq in halves_b:
                        g = g2bufs[hkey][b % NG2]
                        nreg = creg[hkey]
                        nc.gpsimd.reg_load(nreg, cnt_tiles[hkey][0:1, b : b + 1])
                        gi = nc.gpsimd.dma_gather(
                            g[:].rearrange("p (k e) -> p k e", e=TO),
                            p_full[row0 : row0 + HALF, :],
                            idx_t[:, b * K * 8 : (b + 1) * K * 8],
                            K * P,
                            nreg,
                            TO,
                            single_packet=False,
                            queue_num=(2 * b + hq) % NQ,
                        )
                        # pin scheduled order so DMASW lane round-robin
                        # stays aligned with queue_num
                        if prev_gather is not None:
                            add_dep_helper(
                                gi.ins, prev_gather.ins, sync=False,
                                reason="pin swdge lane/queue pairing",
                            )
                        prev_gather = gi
                        for k in range(K):
                            nc.tensor.matmul(
                                out=acc[:],
                                lhsT=s_blk[:, ts(off_h + k, P)],
                                rhs=g[:, ts(k, TO)],
                                start=(i_mm == 0),
                                stop=(i_mm == KT - 1),
                            )
                            i_mm += 1
                    o_sb = opool.tile([P, TO], F32, tag="o")
                    nc.vector.tensor_copy(out=o_sb[:], in_=acc[:])
                    nc.sync.dma_start(out=oshard[ts(b, P), :], in_=o_sb[:])

    nc.compile()
    return nc


# --------------------------------------------------------------------------
# host orchestration
# --------------------------------------------------------------------------
def prep_inputs(x, edge_weight, W1, W2, mask1, mask2, src, dst):
    K_lo, K_hi, per_core = prep_graph(src, dst, edge_weight, x)
    w1 = np.ascontiguousarray(np.asarray(W1, np.float32))
    w2 = np.ascontiguousarray(np.asarray(W2, np.float32))
    m1 = np.ascontiguousarray(
        np.asarray(mask1, np.float32).transpose(1, 0, 2).reshape(IN, T * HID)
    )
    m2 = np.ascontiguousarray(
        np.asarray(mask2, np.float32).transpose(1, 0, 2).reshape(HID, T * OUT)
    )
    iota_h = np.ascontiguousarray(
        np.broadcast_to(np.arange(P, dtype=np.float32), (P, P)).astype(_np_dt())
    )

    in_maps = []
    for c in range(NCORES):
        m = dict(per_core[c])
        m.update(w1=w1, m1=m1, w2=w2, m2=m2, iota=iota_h)
        in_maps.append(m)
    return K_lo, K_hi, per_core, in_maps


def assemble_output(oshards):
    full = np.concatenate(oshards, axis=0)  # [NP_, 512]
    return np.ascontiguousarray(
        full[:N].reshape(N, T, OUT).transpose(1, 0, 2)
    ).astype(np.float32)


def kernel(x, edge_weight, W1, W2, mask1, mask2, src, dst):
    from concourse.bass_utils import run_bass_kernel_spmd

    K_lo, K_hi, per_core, in_maps = prep_inputs(
        x, edge_weight, W1, W2, mask1, mask2, src, dst
    )
    nc = build_kernel_fused(K_lo, K_hi)
    res = run_bass_kernel_spmd(nc, in_maps, core_ids=list(range(NCORES)))
    oshards = [res.results[c]["oshard"] for c in range(NCORES)]
    return assemble_output(oshards)


# revision 13
# speedup vs baseline: 103.1005x; 1.0220x over previous
"""Bass/Trainium2 fused single-launch kernel for nn_BayesianGNN.

Computation (reference):
    agg1 = spmm(x, ew, src, dst)                       # [N, IN]
    for t in range(T):
        h_t   = relu(agg1 @ (W1 * mask1[t]))           # [N, HID]
        agg2  = spmm(h_t, ew, src, dst)                # [N, HID]
        out_t = agg2 @ (W2 * mask2[t])                 # [N, OUT]

Restructure: spmm is linear, so out_t = spmm(relu(agg1 @ W1m_t) @ W2m_t);
all T samples concatenate into P [N, T*OUT=512] so the second spmm runs
once over 512-wide rows.

Single NEFF per core (SPMD over 8 cores, dst-node-sharded):
  phase A: SpMM1 over host-pre-gathered x rows (xg, edge-slot order --
    pure streaming, no on-device gather) x on-device-built one-hot
    segment-sum matrices S -> agg1t in SBUF; dense per-sample MLP ->
    P shard rows, drained to 4 row-slice DRAM buffers.
  sliced AllGather: each slice's collective fires as soon as its rows
    are drained, overlapping the halo exchange with phase A compute.
  phase B: SpMM2: dma_gather of P rows by src (SWDGE) + the same S
    matmuls -> out shard [6272, 512] f32.

S matrices (S[e, j] = w_e * (dst_local_e == j)) are built on device per
block with two DVE ops (iota==dst compare, *w) from 2-byte/edge dst and
weight tables, replacing the 256-byte/edge host s_tab of the previous
version.  Gather tails (padding slots) are never zero-filled: pool
buffers are memset once and padding columns of S carry w=0, so stale
(finite) data in the tail contributes exactly zero.

dma_gather indices are int16, so the P table is split in two halves of
25088 rows; per (block, half) the edge list is padded to a fixed number
of 128-edge chunks (K_lo/K_hi) so the program is identical across all 8
cores (counts/indices/gathered-x differ only as input data).
"""

import sys

if "/opt/trn_rl_repo" not in sys.path:
    sys.path.insert(0, "/opt/trn_rl_repo")

import math

import numpy as np

import concourse.bass as bass
import concourse.tile as tile
from concourse import bacc, mybir
from concourse.bass import ts

F32 = mybir.dt.float32
BF16 = mybir.dt.bfloat16
I16 = mybir.dt.int16
I32 = mybir.dt.int32
USE_BF16 = True
DT = BF16 if USE_BF16 else F32

N, E = 50000, 800000
IN, HID, OUT, T = 96, 128, 64, 8
P = 128  # partitions
NCORES = 8
NBLK = 392  # node blocks of 128
NP_ = NBLK * P  # padded node count 50176
BPC = NBLK // NCORES  # blocks per core = 49
NPC = BPC * P  # nodes per core = 6272
HALF = NP_ // 2  # gather table half size = 25088
TO = T * OUT  # 512
NQ = 4  # SWDGE queues; gather emission order is pinned (nosync chain) so the
# scheduler's DMASW lane round-robin (8 lanes) stays aligned with queue_num%4

# p_int row slices (per-core local row ranges); collectives fire per slice.
SL_START = [0, 1024, 2048, 3072, 4096, 5120, 6144]
SL_LEN = [1024, 1024, 1024, 1024, 1024, 1024, 128]
NSLICE = len(SL_LEN)
# global p_full base row of each slice block
SL_BASE = [0]
for s in range(1, NSLICE):
    SL_BASE.append(SL_BASE[-1] + NCORES * SL_LEN[s - 1])


def _np_dt():
    if USE_BF16:
        import ml_dtypes

        return np.dtype(ml_dtypes.bfloat16)
    return np.dtype(np.float32)


def _prow_of_node():
    """Map global padded node id -> row in the slice-ordered p_full."""
    n = np.arange(NP_, dtype=np.int64)
    c = n // NPC
    r = n % NPC
    s = np.searchsorted(np.asarray(SL_START + [NPC]), r, side="right") - 1
    base = np.asarray(SL_BASE)[s]
    ln = np.asarray(SL_LEN)[s]
    st = np.asarray(SL_START)[s]
    return base + c * ln + (r - st)


# --------------------------------------------------------------------------
# host-side graph prep
# --------------------------------------------------------------------------
def prep_graph(src, dst, ew, x):
    """Partition + pad edges into per-(core, block, half) chunk schedules.

    Returns (K_lo, K_hi, per_core): per_core[c] holds the int16 gather
    index arrays + int32 counts (phase B), bf16 dst/weight tables (S
    build), and the bf16 pre-gathered x rows xg [P, BPC*KT*96] in edge
    slot order (phase A streaming).
    """
    src = np.asarray(src).astype(np.int64).ravel()
    dst = np.asarray(dst).astype(np.int64).ravel()
    ew = np.asarray(ew, dtype=np.float32).ravel()
    x = np.asarray(x, dtype=np.float32)

    prow_map = _prow_of_node()
    pf = prow_map[src]  # p_full row of each edge's source
    half = (pf >= HALF).astype(np.int64)
    blk = dst >> 7
    order = np.lexsort((pf, half, blk))
    sblk = blk[order]
    shalf = half[order]
    ssrc = src[order]
    spf = pf[order]
    sew = ew[order]
    sdl = (dst[order] & 127).astype(np.int64)

    cell = sblk * 2 + shalf
    counts = np.bincount(cell, minlength=NBLK * 2)
    K_lo = int(math.ceil(counts[0::2].max() / P))
    K_hi = int(math.ceil(counts[1::2].max() / P))
    KT = K_lo + K_hi

    cell_starts = np.zeros(NBLK * 2 + 1, np.int64)
    np.cumsum(counts, out=cell_starts[1:])
    pos = np.arange(E, dtype=np.int64) - cell_starts[cell]

    b_local = sblk % BPC
    core = sblk // BPC

    per_core = []
    for c in range(NCORES):
        d = {}
        # slot id within this core: (b_local, k_global, p)
        nslot_t = BPC * KT * P
        slot_src = np.full(nslot_t, -1, np.int64)
        slot_dl = np.zeros(nslot_t, np.int64)
        slot_w = np.zeros(nslot_t, np.float32)
        for s, K, off_h, tag in ((0, K_lo, 0, "lo"), (1, K_hi, K_lo, "hi")):
            m = (core == c) & (shalf == s)
            nslot = BPC * K * P
            idxf = np.full(nslot, -1, np.int16)
            slots_h = b_local[m] * (K * P) + pos[m]
            idxf[slots_h] = (spf[m] - s * HALF).astype(np.int16)
            cnt = np.bincount(b_local[m], minlength=BPC).astype(np.int32)
            # never let a gather have zero valid indices
            empty = cnt == 0
            if empty.any():
                for b in np.nonzero(empty)[0]:
                    idxf[b * K * P] = 0
                cnt[empty] = 1
            d["idx_" + tag] = np.ascontiguousarray(
                np.tile(idxf.reshape(BPC * K * 8, 16).T, (8, 1))
            )
            d["cnt_" + tag] = np.ascontiguousarray(cnt.reshape(1, BPC))
            # global slot ids (k axis combines lo then hi chunks)
            k_idx = pos[m] // P
            e_idx = pos[m] % P
            gslots = (b_local[m] * KT + off_h + k_idx) * P + e_idx
            slot_src[gslots] = ssrc[m]
            slot_dl[gslots] = sdl[m]
            slot_w[gslots] = sew[m]
        # dst/weight tables [P, BPC*KT] (partition = edge-in-chunk)
        d["dsttab"] = np.ascontiguousarray(
            slot_dl.reshape(BPC * KT, P).T.astype(_np_dt())
        )
        d["wtab"] = np.ascontiguousarray(
            slot_w.reshape(BPC * KT, P).T.astype(_np_dt())
        )
        # pre-gathered, edge-weight-scaled x rows [P, BPC*KT*96];
        # padding slots -> zero rows (phase A's S is then a binary one-hot)
        xg = x[np.clip(slot_src, 0, None)] * slot_w[:, None]
        d["xg"] = np.ascontiguousarray(
            xg.reshape(BPC * KT, P, IN)
            .transpose(1, 0, 2)
            .reshape(P, BPC * KT * IN)
            .astype(_np_dt())
        )
        per_core.append(d)
    kmax = {}
    for tag, K in (("lo", K_lo), ("hi", K_hi)):
        allcnt = np.stack([pc["cnt_" + tag][0] for pc in per_core])  # [NCORES, BPC]
        kmax[tag] = [int(v) for v in np.ceil(allcnt.max(axis=0) / P).astype(int)]
    return K_lo, K_hi, kmax, per_core


# --------------------------------------------------------------------------
# fused kernel: SpMM1 + MLP -> sliced AllGather(P) -> SpMM2
# --------------------------------------------------------------------------
def build_kernel_fused(K_lo, K_hi, kmax):
    KT = K_lo + K_hi
    nc = bacc.Bacc(
        "TRN2",
        target_bir_lowering=False,
        debug=False,
        num_devices=NCORES,
        num_swdge_queues=NQ,
        dynamic_dma_scratch_size=1 << 16,
    )

    xg = nc.dram_tensor("xg", [P, BPC * KT * IN], DT, kind="ExternalInput")
    dsttab = nc.dram_tensor("dsttab", [P, BPC * KT], DT, kind="ExternalInput")
    wtab = nc.dram_tensor("wtab", [P, BPC * KT], DT, kind="ExternalInput")
    iota = nc.dram_tensor("iota", [P, P], DT, kind="ExternalInput")
    idx_lo = nc.dram_tensor("idx_lo", [P, BPC * K_lo * 8], I16, kind="ExternalInput")
    idx_hi = nc.dram_tensor("idx_hi", [P, BPC * K_hi * 8], I16, kind="ExternalInput")
    cnt_lo = nc.dram_tensor("cnt_lo", [1, BPC], I32, kind="ExternalInput")
    cnt_hi = nc.dram_tensor("cnt_hi", [1, BPC], I32, kind="ExternalInput")
    w1 = nc.dram_tensor("w1", [IN, HID], F32, kind="ExternalInput")
    m1 = nc.dram_tensor("m1", [IN, T * HID], F32, kind="ExternalInput")
    w2 = nc.dram_tensor("w2", [HID, OUT], F32, kind="ExternalInput")
    m2 = nc.dram_tensor("m2", [HID, T * OUT], F32, kind="ExternalInput")
    oshard = nc.dram_tensor("oshard", [NPC, TO], F32, kind="ExternalOutput")
    p_int_s = [
        nc.dram_tensor(f"p_int{s}", [SL_LEN[s], TO], DT, kind="Internal")
        for s in range(NSLICE)
    ]
    p_full = nc.dram_tensor(
        "p_full", [NP_, TO], DT, kind="Internal", addr_space="Shared"
    )

    with tile.TileContext(nc) as tc:
        with tc.tile_pool(name="const", bufs=1) as cpool:
            # ---- load constants
            def load(t_dram, shape, dtype=F32):
                nm = f"c_{t_dram.name}"
                t_sb = cpool.tile([P, shape[1]], dtype, name=nm, tag=nm)
                nc.sync.dma_start(out=t_sb[: shape[0], :], in_=t_dram[:])
                return t_sb

            iota_t = load(iota, [P, P], DT)
            dst_t = load(dsttab, [P, BPC * KT], DT)
            w_t = load(wtab, [P, BPC * KT], DT)
            idx_lo_t = load(idx_lo, [P, BPC * K_lo * 8], I16)
            idx_hi_t = load(idx_hi, [P, BPC * K_hi * 8], I16)
            cnt_lo_t = load(cnt_lo, [1, BPC], I32)
            cnt_hi_t = load(cnt_hi, [1, BPC], I32)
            creg = {
                "lo": nc.gpsimd.alloc_register("cnt_reg_lo"),
                "hi": nc.gpsimd.alloc_register("cnt_reg_hi"),
            }
            cnt_tiles = {"lo": cnt_lo_t, "hi": cnt_hi_t}
            w1_t = load(w1, [IN, HID])
            m1_t = load(m1, [IN, T * HID])
            w2_t = load(w2, [HID, OUT])
            m2_t = load(m2, [HID, T * OUT])

            # masked weights (only rows :IN of w1m are ever read)
            w1m = cpool.tile([P, T * HID], DT)
            for t in range(T):
                nc.vector.tensor_tensor(
                    out=w1m[:IN, ts(t, HID)],
                    in0=w1_t[:IN, :],
                    in1=m1_t[:IN, ts(t, HID)],
                    op=mybir.AluOpType.mult,
                )
            w2m = cpool.tile([P, T * OUT], DT)
            for t in range(T):
                nc.vector.tensor_tensor(
                    out=w2m[:, ts(t, OUT)],
                    in0=w2_t[:, :],
                    in1=m2_t[:, ts(t, OUT)],
                    op=mybir.AluOpType.mult,
                )

            # agg1 transposed [feat, node] for the whole shard, kept in SBUF
            agg1t = cpool.tile([P, NPC], DT)

            iota_b = iota_t[:].rearrange("p (k j) -> p k j", k=1).broadcast_to(
                [P, KT, P]
            )

            def build_s(spool, eqpool, b, with_w):
                """S_blk [P, KT*P]: S[e, k, j] = (dst[e,k] == j), optionally
                scaled by w[e,k] (phase B; phase A folds w into xg on host)."""
                s_blk = spool.tile([P, KT * P], DT, tag="s_blk", name="s_blk")
                dst_b = (
                    dst_t[:, b * KT : (b + 1) * KT]
                    .rearrange("p (k j) -> p k j", j=1)
                    .broadcast_to([P, KT, P])
                )
                if not with_w:
                    nc.vector.tensor_tensor(
                        out=s_blk[:].rearrange("p (k j) -> p k j", j=P),
                        in0=iota_b,
                        in1=dst_b,
                        op=mybir.AluOpType.is_equal,
                    )
                    return s_blk
                eq = eqpool.tile([P, KT * P], DT, tag="eq", name="eq")
                w_b = (
                    w_t[:, b * KT : (b + 1) * KT]
                    .rearrange("p (k j) -> p k j", j=1)
                    .broadcast_to([P, KT, P])
                )
                nc.vector.tensor_tensor(
                    out=eq[:].rearrange("p (k j) -> p k j", j=P),
                    in0=iota_b,
                    in1=dst_b,
                    op=mybir.AluOpType.is_equal,
                )
                nc.vector.tensor_tensor(
                    out=s_blk[:].rearrange("p (k j) -> p k j", j=P),
                    in0=eq[:].rearrange("p (k j) -> p k j", j=P),
                    in1=w_b,
                    op=mybir.AluOpType.mult,
                )
                return s_blk

            # ---------------- phase A: SpMM1 + dense MLP ----------------
            with (
                tc.tile_pool(name="sA", bufs=3) as sApool,
                tc.tile_pool(name="gA", bufs=3) as gApool,
                tc.tile_pool(name="h", bufs=3) as hpool,
                tc.tile_pool(name="po", bufs=2) as ppool,
                tc.tile_pool(name="acc", bufs=2, space="PSUM") as acc_pool,
                tc.tile_pool(name="ph", bufs=2, space="PSUM") as ph_pool,
                tc.tile_pool(name="pp", bufs=1, space="PSUM") as pp_pool,
            ):

                def spmm1_block(b):
                    g = gApool.tile([P, KT * IN], DT, tag="gA", name="gA")
                    nc.sync.dma_start(
                        out=g[:], in_=xg[:, b * KT * IN : (b + 1) * KT * IN]
                    )
                    s_blk = build_s(sApool, None, b, with_w=False)
                    acc = acc_pool.tile([P, P], F32, space="PSUM", tag="acc", name="acc")
                    ks = list(range(kmax["lo"][b])) + [
                        K_lo + k for k in range(kmax["hi"][b])
                    ]
                    for i, k in enumerate(ks):
                        nc.tensor.matmul(
                            out=acc[:IN, :],
                            lhsT=g[:, ts(k, IN)],
                            rhs=s_blk[:, ts(k, P)],
                            start=(i == 0),
                            stop=(i == len(ks) - 1),
                        )
                    nc.vector.tensor_copy(out=agg1t[:IN, ts(b, P)], in_=acc[:IN, :])

                def mlp_group(off, w_):
                    nj = w_ // P
                    psum_p = [
                        pp_pool.tile([P, TO], F32, space="PSUM", tag=f"pp{j}", name=f"pp{j}")
                        for j in range(nj)
                    ]
                    for t in range(T):
                        psum_h = ph_pool.tile([P, w_], F32, space="PSUM", tag="ph", name="ph")
                        nc.tensor.matmul(
                            out=psum_h[:],
                            lhsT=w1m[:IN, ts(t, HID)],
                            rhs=agg1t[:IN, off : off + w_],
                            start=True,
                            stop=True,
                        )
                        h_sb = hpool.tile([P, w_], DT, tag="h", name="h_sb")
                        nc.scalar.activation(
                            out=h_sb[:],
                            in_=psum_h[:],
                            func=mybir.ActivationFunctionType.Relu,
                        )
                        for j in range(nj):
                            nc.tensor.matmul(
                                out=psum_p[j][:, ts(t, OUT)],
                                lhsT=h_sb[:, ts(j, P)],
                                rhs=w2m[:, ts(t, OUT)],
                                start=True,
                                stop=True,
                            )
                    # drain to the slice buffer this group belongs to
                    s = max(i for i in range(NSLICE) if SL_START[i] <= off)
                    for j in range(nj):
                        p_sb = ppool.tile([P, TO], DT, tag="po", name="p_sb")
                        nc.scalar.copy(out=p_sb[:], in_=psum_p[j][:])
                        r0 = off + j * P - SL_START[s]
                        nc.sync.dma_start(
                            out=p_int_s[s][r0 : r0 + P, :], in_=p_sb[:]
                        )

                groups = []
                off = 0
                while off < NPC:
                    w_ = min(512, NPC - off)
                    groups.append((off, w_))
                    off += w_
                b = 0
                done_slices = 0
                for off, w_ in groups:
                    while b * P < off + w_:
                        spmm1_block(b)
                        b += 1
                    mlp_group(off, w_)
                    # fire the collective for any slice fully drained
                    while (
                        done_slices < NSLICE
                        and off + w_ >= SL_START[done_slices] + SL_LEN[done_slices]
                    ):
                        s = done_slices
                        nc.gpsimd.collective_compute(
                            "AllGather",
                            mybir.AluOpType.bypass,
                            replica_groups=[list(range(NCORES))],
                            ins=[p_int_s[s][:].opt()],
                            outs=[
                                p_full[
                                    SL_BASE[s] : SL_BASE[s] + NCORES * SL_LEN[s]
                                ].opt()
                            ],
                        )
                        done_slices += 1
                assert done_slices == NSLICE

            # ---------------- phase B: SpMM2 over P ----------------
            with (
                tc.tile_pool(name="sB", bufs=3) as sBpool,
                tc.tile_pool(name="eqB", bufs=2) as eqBpool,
                tc.tile_pool(name="o", bufs=3) as opool,
                tc.tile_pool(name="po2", bufs=4, space="PSUM") as po2_pool,
            ):
                halves_b = (
                    (K_lo, 0, idx_lo_t, "lo", 0, 0),
                    (K_hi, HALF, idx_hi_t, "hi", K_lo, 1),
                )
                # persistent gather buffers, memset once: gather tails
                # (slots beyond cnt) keep the zeros / stale finite rows,
                # which S's w=0 padding columns nullify in the matmul.
                NG2 = 4  # buffer cycle must align with the queue cycle (sem-lane binding)
                g2bufs = {}
                for K, row0, idx_t, hkey, off_h, hq in halves_b:
                    g2bufs[hkey] = [
                        cpool.tile([P, K * TO], DT, name=f"g2{hkey}{i}", tag=f"g2{hkey}{i}")
                        for i in range(NG2)
                    ]
                    for g in g2bufs[hkey]:
                        nc.vector.memset(g[:], 0.0)
                from concourse.tile_rust import add_dep_helper

                prev_gather = None
                for b in range(BPC):
                    s_blk = build_s(sBpool, eqBpool, b, with_w=True)
                    acc = po2_pool.tile([P, TO], F32, space="PSUM", tag="acc2")
                    n_mm = kmax["lo"][b] + kmax["hi"][b]
                    i_mm = 0
                    for K, row0, idx_t, hkey, off_h, hq in halves_b:
                        g = g2bufs[hkey][b % NG2]
                        nreg = creg[hkey]
                        nc.gpsimd.reg_load(nreg, cnt_tiles[hkey][0:1, b : b + 1])
                        gi = nc.gpsimd.dma_gather(
                            g[:].rearrange("p (k e) -> p k e", e=TO),
                            p_full[row0 : row0 + HALF, :],
                            idx_t[:, b * K * 8 : (b + 1) * K * 8],
                            K * P,
                            nreg,
                            TO,
                            single_packet=False,
                            queue_num=(2 * b + hq) % NQ,
                        )
                        # pin scheduled order so DMASW lane round-robin
                        # stays aligned with queue_num
                        if prev_gather is not None:
                            add_dep_helper(
                                gi.ins, prev_gather.ins, sync=False,
                                reason="pin swdge lane/queue pairing",
                            )
                        prev_gather = gi
                        for k in range(kmax[hkey][b]):
                            nc.tensor.matmul(
                                out=acc[:],
                                lhsT=s_blk[:, ts(off_h + k, P)],
                                rhs=g[:, ts(k, TO)],
                                start=(i_mm == 0),
                                stop=(i_mm == n_mm - 1),
                            )
                            i_mm += 1
                    o_sb = opool.tile([P, TO], F32, tag="o")
                    nc.vector.tensor_copy(out=o_sb[:], in_=acc[:])
                    nc.sync.dma_start(out=oshard[ts(b, P), :], in_=o_sb[:])

    nc.compile()
    return nc


# --------------------------------------------------------------------------
# host orchestration
# --------------------------------------------------------------------------
def prep_inputs(x, edge_weight, W1, W2, mask1, mask2, src, dst):
    K_lo, K_hi, kmax, per_core = prep_graph(src, dst, edge_weight, x)
    w1 = np.ascontiguousarray(np.asarray(W1, np.float32))
    w2 = np.ascontiguousarray(np.asarray(W2, np.float32))
    m1 = np.ascontiguousarray(
        np.asarray(mask1, np.float32).transpose(1, 0, 2).reshape(IN, T * HID)
    )
    m2 = np.ascontiguousarray(
        np.asarray(mask2, np.float32).transpose(1, 0, 2).reshape(HID, T * OUT)
    )
    iota_h = np.ascontiguousarray(
        np.broadcast_to(np.arange(P, dtype=np.float32), (P, P)).astype(_np_dt())
    )

    in_maps = []
    for c in range(NCORES):
        m = dict(per_core[c])
        m.update(w1=w1, m1=m1, w2=w2, m2=m2, iota=iota_h)
        in_maps.append(m)
    return K_lo, K_hi, kmax, per_core, in_maps


def assemble_output(oshards):
    full = np.concatenate(oshards, axis=0)  # [NP_, 512]
    return np.ascontiguousarray(
        full[:N].reshape(N, T, OUT).transpose(1, 0, 2)
    ).astype(np.float32)


def kernel(x, edge_weight, W1, W2, mask1, mask2, src, dst):
    from concourse.bass_utils import run_bass_kernel_spmd

    K_lo, K_hi, kmax, per_core, in_maps = prep_inputs(
        x, edge_weight, W1, W2, mask1, mask2, src, dst
    )
    nc = build_kernel_fused(K_lo, K_hi, kmax)
    res = run_bass_kernel_spmd(nc, in_maps, core_ids=list(range(NCORES)))
    oshards = [res.results[c]["oshard"] for c in range(NCORES)]
    return assemble_output(oshards)
